# revision 12
# baseline (speedup 1.0000x reference)
"""Multi-head causal self-attention (B=2, S=2048, D=1024, H=16) on 8 TRN2 cores.

Sharding: core = b*4 + hg  (b in {0,1} batch, hg in {0..3} head-group of 4 heads).
Per core: project qT/kT (pair-packed [128, S], fp16) and v ([S, 64] blocks, fp16),
compute transposed scores S^T = K Q^T per head (k on partitions), exp on ScalarE
(fp16 out, both heads in one strided activation), causal diag masking via one
fused strided tensor_mul against a duplicated upper-tri matrix, PV matmul with a
ones-column appended to V so row 64 of the accumulator is the softmax sum.
Normalization: Pool-engine copy of the [65, qcw] accumulator out of PSUM,
reciprocal on VectorE, partition-broadcast on Pool, one fused multiply+cast on
VectorE. Output projection partials stream to HBM as fp16; host sums the 4
per-batch partials and adds (b_v @ w_o.T + b_o); b_k is dropped (softmax is
invariant to per-query constants); b_q is applied on-device. All matmul operands
are fp16 (same PE rate as bf16, 8x the mantissa); accumulation is fp32 in PSUM.
Startup: x is DMAed in two column-chunks per 128-row tile and all hot-path DMAs
are triggered from the Pool queue (25ns/trigger vs 565ns on SP), so the first
projection chain starts ~1us in. The last pair-1 chunk is split 512->256+256 to
shrink the end-of-kernel norm+projection tail.
"""

import numpy as np
import ml_dtypes

import concourse.bass as bass
import concourse.mybir as mybir
import concourse.tile as tile
from concourse import bacc
from concourse.bass_utils import run_bass_kernel_spmd

B, S, D, H, DK = 2, 2048, 1024, 16, 64
N_CORES = 8
F32 = mybir.dt.float32
F16 = mybir.dt.float16
NPF16 = np.float16
AF = mybir.ActivationFunctionType

# chunk lists per pair: (q0, width)
CHUNKS = [
    [(0, 512), (512, 512), (1024, 512), (1536, 512)],
    [(0, 512), (512, 512), (1024, 512), (1536, 256), (1792, 256)],
]


def _build(debug=False):
    nc = bacc.Bacc("TRN2", target_bir_lowering=False, debug=False,
                   num_devices=N_CORES)
    xT = nc.dram_tensor("xT", [D, S], F16, kind="ExternalInput").ap()
    wqT = nc.dram_tensor("wqT", [D, 256], F16, kind="ExternalInput").ap()
    wkT = nc.dram_tensor("wkT", [D, 256], F16, kind="ExternalInput").ap()
    wvT = nc.dram_tensor("wvT", [D, 256], F16, kind="ExternalInput").ap()
    woT = nc.dram_tensor("woT", [256, D], F16, kind="ExternalInput").ap()
    bq2 = nc.dram_tensor("bq2", [128, 2], F32, kind="ExternalInput").ap()
    tri2 = nc.dram_tensor("tri2", [128, 256], F16, kind="ExternalInput").ap()
    y = nc.dram_tensor("y", [S, D], F16, kind="ExternalOutput").ap()
    dbg = {}
    if debug:
        for nm, shp in [("qT", [128, 2, S]), ("kT", [128, 2, S]),
                        ("vv", [128, 16, 260]), ("oT", [128, 2, S])]:
            dbg[nm] = nc.dram_tensor(nm, shp, F16, kind="ExternalOutput").ap()

    NKT = S // 128   # k tiles

    with tile.TileContext(nc) as tc, \
            nc.allow_low_precision(reason="fp16 attention kernel"):
        with (
            tc.tile_pool(name="persist", bufs=1) as persist,
            tc.tile_pool(name="kqv", bufs=2) as kqv,
        ):
            qT_sb = [kqv.tile([128, S], F16, tag="qT", name=f"qT{p}") for p in range(2)]
            kT_sb = [kqv.tile([128, S], F16, tag="kT", name=f"kT{p}") for p in range(2)]
            v_sb = [persist.tile([128, 4 * 65], F16, tag=f"v{t}", name=f"v{t}")
                    for t in range(NKT)]
            outT_sb = [persist.tile([128, S], F16, tag=f"oT{p}", name=f"oTs{p}")
                       for p in range(2)]
            wo_sb = [persist.tile([128, D], F16, tag=f"wo{p}", name=f"wo{p}")
                     for p in range(2)]
            tri_sb = persist.tile([128, 256], F16, tag="tri")
            bq_sb = persist.tile([128, 2], F32, tag="bq")
            tri_v = tri_sb.rearrange("p (c w) -> p c w", c=2)

            with (
                tc.tile_pool(name="xw", bufs=1) as xw,
                tc.tile_pool(name="ep", bufs=4) as ep,
                tc.tile_pool(name="rp", bufs=6) as rp,
            ):
                xt0 = [xw.tile([128, 512], F16, tag=f"x0{c}", name=f"xt0{c}")
                       for c in range(8)]
                xt1 = [xw.tile([128, 1536], F16, tag=f"x1{c}", name=f"xt1{c}")
                       for c in range(8)]
                wq_sb = [xw.tile([128, 256], F16, tag=f"wq{c}", name=f"wqs{c}") for c in range(8)]
                wk_sb = [xw.tile([128, 256], F16, tag=f"wk{c}", name=f"wks{c}") for c in range(8)]
                wv_sb = [xw.tile([128, 256], F16, tag=f"wv{c}", name=f"wvs{c}") for c in range(8)]

                # hot-path DMAs on the Pool queue: 25ns per trigger
                nc.gpsimd.dma_start(out=bq_sb, in_=bq2)
                nc.gpsimd.dma_start(out=tri_sb, in_=tri2)
                for c in range(8):
                    nc.gpsimd.dma_start(out=xt0[c], in_=xT[c * 128:(c + 1) * 128, 0:512])
                for c in range(8):
                    nc.gpsimd.dma_start(out=wq_sb[c], in_=wqT[c * 128:(c + 1) * 128, :])
                for c in range(8):
                    nc.gpsimd.dma_start(out=wk_sb[c], in_=wkT[c * 128:(c + 1) * 128, :])
                for c in range(8):
                    nc.gpsimd.dma_start(out=wv_sb[c], in_=wvT[c * 128:(c + 1) * 128, :])
                # bulk x columns + output weights on the SP queue
                for c in range(8):
                    nc.sync.dma_start(out=xt1[c], in_=xT[c * 128:(c + 1) * 128, 512:S])
                for p in range(2):
                    nc.sync.dma_start(out=wo_sb[p], in_=woT[p * 128:(p + 1) * 128, :])

                def xcols(c, a, b):
                    # view of x tile c, columns [a, b)
                    if b <= 512:
                        return xt0[c][:, a:b]
                    assert a >= 512
                    return xt1[c][:, a - 512:b - 512]

                def qk_chain(p, j, which, pool):
                    ps = pool.tile([128, 512], F32, tag="proj", name="ps")
                    w_sb = wq_sb if which == "q" else wk_sb
                    for c in range(8):
                        nc.tensor.matmul(
                            ps, w_sb[c][:, p * 128:(p + 1) * 128],
                            xcols(c, j * 512, (j + 1) * 512),
                            start=(c == 0), stop=(c == 7))
                    if which == "q":
                        nc.vector.tensor_scalar_add(
                            qT_sb[p][:, j * 512:(j + 1) * 512], ps, bq_sb[:, p:p + 1])
                    else:
                        nc.vector.tensor_copy(kT_sb[p][:, j * 512:(j + 1) * 512], ps)

                def v_chain(t, pool):
                    ps_v = pool.tile([128, 256], F32, tag="proj", name="ps_v")
                    for c in range(8):
                        nc.tensor.matmul(
                            ps_v, xcols(c, t * 128, (t + 1) * 128), wv_sb[c],
                            start=(c == 0), stop=(c == 7))
                    v_view = v_sb[t].rearrange("p (h w) -> p h w", w=65)
                    nc.vector.memset(v_view[:, :, 64:65], 1.0)
                    nc.vector.tensor_copy(
                        v_view[:, :, 0:64],
                        ps_v.rearrange("p (h w) -> p h w", w=64))

                norm_rest = []

                def emit_norm(p_, q0_, qcw_, o_ps_):
                    for s in range(2):
                        oc = rp.tile([65, 512], F32, tag="oc", name="o_cp",
                                     bufs=4)
                        nc.vector.tensor_copy(oc[:, 0:qcw_], o_ps_[s][:, 0:qcw_])
                        norm_rest.append(
                            lambda p_=p_, q0_=q0_, qcw_=qcw_, s=s, oc=oc:
                            finish_norm(p_, q0_, qcw_, s, oc))

                def finish_norm(p_, q0_, qcw_, s, oc):
                    # reciprocal_approx_fast mis-executes on HW when its input
                    # sits at a partition offset; align sums to partition 0
                    sums = rp.tile([1, 512], F32, tag="sums", name="sums")
                    nc.vector.tensor_copy(sums[:, 0:qcw_], oc[64:65, 0:qcw_])
                    rec = rp.tile([1, 512], F32, tag="rec", name="recip")
                    nc.vector.reciprocal_approx_fast(
                        out=rec[:, 0:qcw_], in_=sums[:, 0:qcw_])
                    bc = rp.tile([64, 512], F32, tag="bc", name="bc")
                    nc.gpsimd.partition_broadcast(bc[:, 0:qcw_], rec[:, 0:qcw_])
                    nc.vector.tensor_mul(
                        outT_sb[p_][s * 64:(s + 1) * 64, q0_:q0_ + qcw_],
                        oc[0:64, 0:qcw_], bc[:, 0:qcw_])

                def emit_pair(p, fillers):
                    for ci, (q0, qcw) in enumerate(CHUNKS[p]):
                        nkt = (q0 + qcw) // 128
                        o_ps = [opp.tile([65, 512], F32, tag=f"o{s}", name=f"ops{s}")
                                for s in range(2)]
                        pend = None
                        for kt in range(nkt):
                            o = kt * 128 - q0
                            diag = o >= 0
                            lo = o if diag else 0
                            s_ab = sqp.tile([128, 1024], F32, tag="sq", name="s_ab")
                            s_v = s_ab.rearrange("p (c w) -> p c w", c=2)
                            for s in range(2):
                                nc.tensor.matmul(
                                    s_v[:, s, lo:qcw],
                                    kT_sb[p][s * 64:(s + 1) * 64,
                                             kt * 128:(kt + 1) * 128],
                                    qT_sb[p][s * 64:(s + 1) * 64,
                                             q0 + lo:q0 + qcw],
                                    start=True, stop=True,
                                    tile_position=(s * 64, 0),
                                    skip_group_check=True)
                            e_ab = ep.tile([128, 1024], F16, tag="e", name="e_ab")
                            e_v = e_ab.rearrange("p (c w) -> p c w", c=2)
                            for s in range(2):
                                nc.scalar.activation(
                                    e_v[:, s, lo:qcw], s_v[:, s, lo:qcw],
                                    AF.Exp, scale=0.125)
                            if diag:
                                nc.vector.tensor_mul(
                                    e_v[:, :, o:o + 128], e_v[:, :, o:o + 128],
                                    tri_v)
                            if kt == 0:
                                while norm_rest:
                                    norm_rest.pop(0)()
                            if fillers is not None:
                                fillers(ci, kt, nkt)
                            if pend is not None:
                                _kt, _e, _lo = pend
                                for s in range(2):
                                    hb = 2 * p + s
                                    nc.tensor.matmul(
                                        o_ps[s][:, _lo:qcw],
                                        v_sb[_kt][:, hb * 65:(hb + 1) * 65],
                                        _e[:, s, _lo:qcw],
                                        start=(_kt == 0), stop=False,
                                        skip_group_check=True)
                            pend = (kt, e_v, lo)
                        _kt, _e, _lo = pend
                        for s in range(2):
                            hb = 2 * p + s
                            nc.tensor.matmul(
                                o_ps[s][:, _lo:qcw],
                                v_sb[_kt][:, hb * 65:(hb + 1) * 65],
                                _e[:, s, _lo:qcw],
                                start=False, stop=True,
                                skip_group_check=True)
                        emit_norm(p, q0, qcw, o_ps)
                        yield ci
                    while norm_rest:
                        norm_rest.pop(0)()

                # ---- pair 0 front: j0 q/k + v0-3 up front (after chunk-0 x) ----
                with tc.tile_pool(name="ppsA", bufs=2, space="PSUM") as ppsA:
                    ps_q0 = ppsA.tile([128, 512], F32, tag="projA", name="ps_q0")
                    ps_k0 = ppsA.tile([128, 512], F32, tag="projA", name="ps_k0")
                    # ordered to match DMA arrival: wq -> wk (v0-3 run as
                    # chunk-0 fillers, overlapping the first scores/exp)
                    for c in range(8):
                        nc.tensor.matmul(
                            ps_q0, wq_sb[c][:, 0:128], xt0[c],
                            start=(c == 0), stop=(c == 7))
                    for c in range(8):
                        nc.tensor.matmul(
                            ps_k0, wk_sb[c][:, 0:128], xt0[c],
                            start=(c == 0), stop=(c == 7))
                    nc.vector.tensor_scalar_add(
                        qT_sb[0][:, 0:512], ps_q0, bq_sb[:, 0:1])
                    nc.vector.tensor_copy(kT_sb[0][:, 0:512], ps_k0)
                with (
                    tc.tile_pool(name="sq", bufs=2, space="PSUM") as sqp,
                    tc.tile_pool(name="ops", bufs=1, space="PSUM") as opp,
                ):
                    with tc.tile_pool(name="pps", bufs=2, space="PSUM") as pps:
                        fillers = []
                        for j in range(1, 4):
                            fillers.append(lambda j=j: qk_chain(0, j, "q", pps))
                            fillers.append(lambda j=j: qk_chain(0, j, "k", pps))
                            for t in range(4 * j, 4 * j + 4):
                                fillers.append(lambda t=t: v_chain(t, pps))
                        # chunk 0 gets v0-3 + j1 + v4-7, chunk 1 j2+v8-11,
                        # chunk 2 j3+v12-15, chunk 3 all 8 pair-1 projections
                        v03 = [lambda t=t: v_chain(t, pps) for t in range(4)]
                        sched = {0: v03 + fillers[0:6], 1: fillers[6:12],
                                 2: fillers[12:18]}
                        sched[3] = []
                        for j in range(4):
                            sched[3].append(lambda j=j: qk_chain(1, j, "q", pps))
                            sched[3].append(lambda j=j: qk_chain(1, j, "k", pps))
                        queues = [list(sched.get(ci, [])) for ci in range(4)]

                        def filler_pop(ci, kt, nkt):
                            q = queues[ci]
                            rem_slots = nkt - kt
                            while q and len(q) >= rem_slots:
                                q.pop(0)()
                            if q:
                                q.pop(0)()

                        for _ci in emit_pair(0, filler_pop):
                            while queues[_ci]:
                                queues[_ci].pop(0)()

                    if debug:
                        for p in range(2):
                            nc.sync.dma_start(out=dbg["qT"][:, p, :], in_=qT_sb[p])
                            nc.sync.dma_start(out=dbg["kT"][:, p, :], in_=kT_sb[p])
                        for t in range(NKT):
                            nc.sync.dma_start(out=dbg["vv"][:, t, :], in_=v_sb[t])

                    # ---- pair 1: output projection as fillers ----
                    with tc.tile_pool(name="fps", bufs=2, space="PSUM") as fps:
                        with tc.tile_pool(name="fsb", bufs=4) as fsb:
                            c_alt = [0]

                            def c_unit(qt, oc_i, vec=False):
                                f_ps = fps.tile([128, 512], F32, tag="f", name="f_ps")
                                for p in range(2):
                                    nc.tensor.matmul(
                                        f_ps, outT_sb[p][:, qt * 128:(qt + 1) * 128],
                                        wo_sb[p][:, oc_i * 512:(oc_i + 1) * 512],
                                        start=(p == 0), stop=(p == 1))
                                f_sb = fsb.tile([128, 512], F16, tag="f", name="f_sb")
                                c_alt[0] ^= 1
                                if vec or c_alt[0]:
                                    nc.vector.tensor_copy(f_sb, f_ps)
                                else:
                                    nc.scalar.activation(f_sb, f_ps, AF.Identity)
                                nc.gpsimd.dma_start(
                                    out=y[qt * 128:(qt + 1) * 128,
                                          oc_i * 512:(oc_i + 1) * 512],
                                    in_=f_sb)

                            NCH = len(CHUNKS[1])
                            cqueues = [[] for _ in range(NCH)]

                            def c_pop(ci, kt, nkt):
                                q = cqueues[ci]
                                rem_slots = nkt - kt
                                while q and len(q) >= rem_slots:
                                    q.pop(0)()
                                if q:
                                    q.pop(0)()

                            qt_ranges = [(0, 4), (4, 8), (8, 12), (12, 14), (14, 16)]
                            for ci in emit_pair(1, c_pop):
                                a, b = qt_ranges[ci]
                                vec = ci >= 3
                                units = []
                                for qt in range(a, b):
                                    for oc_i in range(2):
                                        units.append(
                                            lambda qt=qt, oc_i=oc_i, vec=vec:
                                            c_unit(qt, oc_i, vec))
                                if ci < NCH - 1:
                                    cqueues[ci + 1].extend(units)
                                else:
                                    while norm_rest:
                                        norm_rest.pop(0)()
                                    for u in units:
                                        u()
                            for q in cqueues:
                                while q:
                                    q.pop(0)()

            if debug:
                for p in range(2):
                    nc.sync.dma_start(out=dbg["oT"][:, p, :], in_=outT_sb[p])

    nc.compile()
    return nc


_cached = {}


def _get_nc(debug=False):
    key = bool(debug)
    if key not in _cached:
        _cached[key] = _build(debug)
    return _cached[key]


def _prep_inputs(x, w_q, b_q, w_k, w_v):
    tri = np.triu(np.ones((128, 128), np.float32)).astype(NPF16)
    tri2 = np.concatenate([tri, tri], axis=1)
    wqT_f = np.ascontiguousarray(w_q.T).astype(NPF16)
    wkT_f = np.ascontiguousarray(w_k.T).astype(NPF16)
    wvT_f = np.ascontiguousarray(w_v.T).astype(NPF16)
    in_maps = []
    for core in range(N_CORES):
        b, hg = divmod(core, 4)
        cs = slice(hg * 256, (hg + 1) * 256)
        in_maps.append({
            "xT": np.ascontiguousarray(x[b].T).astype(NPF16),
            "wqT": np.ascontiguousarray(wqT_f[:, cs]),
            "wkT": np.ascontiguousarray(wkT_f[:, cs]),
            "wvT": np.ascontiguousarray(wvT_f[:, cs]),
            "bq2": np.ascontiguousarray(
                b_q[hg * 256:(hg + 1) * 256].reshape(2, 128).T.astype(np.float32)),
            "tri2": tri2,
        })
    return in_maps


def _numpy_reference(x, attention_mask, w_q, b_q, w_k, b_k, w_v, b_v, w_o, b_o):
    x = x.astype(np.float64)
    q = (x @ w_q.T + b_q).reshape(B, S, H, DK).transpose(0, 2, 1, 3)
    k = (x @ w_k.T + b_k).reshape(B, S, H, DK).transpose(0, 2, 1, 3)
    v = (x @ w_v.T + b_v).reshape(B, S, H, DK).transpose(0, 2, 1, 3)
    scores = np.einsum("bhqd,bhkd->bhqk", q, k, optimize=True) / np.sqrt(DK)
    causal = np.tril(np.ones((S, S), bool))
    mask = causal[None, None] & (attention_mask[:, None, None, :] != 0)
    scores = np.where(mask, scores, -np.inf)
    scores -= scores.max(-1, keepdims=True)
    e = np.exp(scores)
    attn = e / e.sum(-1, keepdims=True)
    out = np.einsum("bhqk,bhkd->bhqd", attn, v, optimize=True)
    out = out.transpose(0, 2, 1, 3).reshape(B, S, D)
    return (out @ w_o.T + b_o).astype(np.float32)


def kernel(x, attention_mask, w_q, b_q, w_k, b_k, w_v, b_v, w_o, b_o,
           _debug=False, _trace=False):
    x = np.asarray(x, np.float32)
    attention_mask = np.asarray(attention_mask)
    if not np.all(attention_mask != 0):
        return _numpy_reference(np.asarray(x), np.asarray(attention_mask),
                                *[np.asarray(a) for a in
                                  (w_q, b_q, w_k, b_k, w_v, b_v, w_o, b_o)])
    w_q, w_k, w_v, w_o = [np.asarray(w, np.float32) for w in (w_q, w_k, w_v, w_o)]
    b_q, b_k, b_v, b_o = [np.asarray(b, np.float32) for b in (b_q, b_k, b_v, b_o)]

    nc = _get_nc(_debug)
    in_maps = _prep_inputs(x, w_q, b_q, w_k, w_v)
    woT_f = np.ascontiguousarray(w_o.T).astype(NPF16)
    for core in range(N_CORES):
        hg = core % 4
        in_maps[core]["woT"] = np.ascontiguousarray(
            woT_f[hg * 256:(hg + 1) * 256, :])

    res = run_bass_kernel_spmd(nc, in_maps, list(range(N_CORES)), trace=_trace)
    const_row = (b_v @ w_o.T + b_o).astype(np.float32)
    y = np.zeros((B, S, D), np.float32)
    for core in range(N_CORES):
        b = core // 4
        y[b] += res.results[core]["y"].astype(np.float32)
    y += const_row
    if _debug or _trace:
        return y, res
    return y


# revision 19
# speedup vs baseline: 1.1864x; 1.1864x over previous
"""Multi-head causal self-attention (B=2, S=2048, D=1024, H=16) on 8 TRN2 cores.

Sharding: core = b*4 + hg  (b in {0,1} batch, hg in {0..3} head-group of 4 heads).
Per core: project qT/kT (pair-packed [128, S], fp16) and v ([S, 64] blocks, fp16),
compute transposed scores S^T = K Q^T per head (k on partitions, two heads per
k-tile via tile_position), exp on ScalarE (both heads in one strided activation,
fp16 out), causal diag masking via one fused strided tensor_mul against a
duplicated upper-tri matrix, PV matmul with a ones-column appended to V so row 64
of the accumulator is the softmax sum. Normalization per chunk: copy the value
rows and the sums row out of PSUM (sums aligned to partition 0 — the DVE
reciprocal mis-executes on HW with a partition-offset input), one reciprocal +
one Q7 partition-broadcast for both heads, then a fused multiply+cast per head.
Output projection partials stream to HBM as fp16; host sums the 4 per-batch
partials and adds (b_v @ w_o.T + b_o); b_k is dropped (softmax is invariant to
per-query constants); b_q is applied on-device. All matmul operands are fp16
(same PE rate as bf16, 8x the mantissa); accumulation is fp32 in PSUM.
Startup: inputs arrive as a handful of large rearranged DMAs split across the
Pool-triggered ring (x first-chunk + QKV weights; 25ns/trigger) and the SP ring
(x tail columns + output weights), so the first projection chain starts ~2us in
and everything else streams in behind compute.
"""

import numpy as np
import ml_dtypes

import concourse.bass as bass
import concourse.mybir as mybir
import concourse.tile as tile
from concourse import bacc
from concourse.bass_utils import run_bass_kernel_spmd

B, S, D, H, DK = 2, 2048, 1024, 16, 64
N_CORES = 8
F32 = mybir.dt.float32
F16 = mybir.dt.float16
NPF16 = np.float16
AF = mybir.ActivationFunctionType

CHUNKS = [
    [(0, 512), (512, 512), (1024, 512), (1536, 512)],
    [(0, 512), (512, 512), (1024, 512), (1536, 512)],
]


def _build(debug=False):
    nc = bacc.Bacc("TRN2", target_bir_lowering=False, debug=False,
                   num_devices=N_CORES)
    xT = nc.dram_tensor("xT", [D, S], F16, kind="ExternalInput").ap()
    wqT = nc.dram_tensor("wqT", [D, 256], F16, kind="ExternalInput").ap()
    wkT = nc.dram_tensor("wkT", [D, 256], F16, kind="ExternalInput").ap()
    wvT = nc.dram_tensor("wvT", [D, 256], F16, kind="ExternalInput").ap()
    woT = nc.dram_tensor("woT", [256, D], F16, kind="ExternalInput").ap()
    bq2 = nc.dram_tensor("bq2", [128, 2], F32, kind="ExternalInput").ap()
    tri2 = nc.dram_tensor("tri2", [128, 256], F16, kind="ExternalInput").ap()
    y = nc.dram_tensor("y", [S, D], F16, kind="ExternalOutput").ap()
    dbg = {}
    if debug:
        for nm, shp in [("qT", [128, 2, S]), ("kT", [128, 2, S]),
                        ("vv", [128, 16, 260]), ("oT", [128, 2, S])]:
            dbg[nm] = nc.dram_tensor(nm, shp, F16, kind="ExternalOutput").ap()

    NKT = S // 128   # k tiles
    # c-block views of the DRAM inputs: row (c*128+p) -> [p, c, w]
    xT_r = xT.rearrange("(c p) w -> p c w", p=128)
    wqT_r = wqT.rearrange("(c p) w -> p c w", p=128)
    wkT_r = wkT.rearrange("(c p) w -> p c w", p=128)
    wvT_r = wvT.rearrange("(c p) w -> p c w", p=128)

    with tile.TileContext(nc) as tc, \
            nc.allow_low_precision(reason="fp16 attention kernel"):
        with (
            tc.tile_pool(name="persist", bufs=1) as persist,
            tc.tile_pool(name="kqv", bufs=2) as kqv,
        ):
            qT_sb = [kqv.tile([128, S], F16, tag="qT", name=f"qT{p}") for p in range(2)]
            kT_sb = [kqv.tile([128, S], F16, tag="kT", name=f"kT{p}") for p in range(2)]
            v_sb = [persist.tile([128, 4 * 65], F16, tag=f"v{t}", name=f"v{t}")
                    for t in range(NKT)]
            outT_sb = [persist.tile([128, S], F16, tag=f"oT{p}", name=f"oTs{p}")
                       for p in range(2)]
            wo_sb = [persist.tile([128, D], F16, tag=f"wo{p}", name=f"wo{p}")
                     for p in range(2)]
            tri_sb = persist.tile([128, 256], F16, tag="tri")
            bq_sb = persist.tile([128, 2], F32, tag="bq")
            tri_v = tri_sb.rearrange("p (c w) -> p c w", c=2)

            with (
                tc.tile_pool(name="xw", bufs=1) as xw,
                tc.tile_pool(name="ep", bufs=4) as ep,
                tc.tile_pool(name="rp", bufs=4) as rp,
            ):
                xt0_t = xw.tile([128, 8 * 512], F16, tag="x0", name="xt0")
                xt1_t = xw.tile([128, 8 * 1536], F16, tag="x1", name="xt1")
                wq_t = xw.tile([128, 8 * 256], F16, tag="wq", name="wqs")
                wk_t = xw.tile([128, 8 * 256], F16, tag="wk", name="wks")
                wv_t = xw.tile([128, 8 * 256], F16, tag="wv", name="wvs")
                xt0 = xt0_t.rearrange("p (c w) -> p c w", c=8)
                xt1 = xt1_t.rearrange("p (c w) -> p c w", c=8)
                wq_v = wq_t.rearrange("p (c w) -> p c w", c=8)
                wk_v = wk_t.rearrange("p (c w) -> p c w", c=8)
                wv_v = wv_t.rearrange("p (c w) -> p c w", c=8)

                # Single need-ordered Pool ring (25ns/trigger): the HBM wire
                # (~300GB/s) is the startup constraint, so transfers must
                # land in exactly the order compute consumes them.
                nc.gpsimd.dma_start(out=bq_sb, in_=bq2)
                nc.gpsimd.dma_start(out=tri_sb, in_=tri2)
                nc.gpsimd.dma_start(out=xt0[:, 0:4, :], in_=xT_r[:, 0:4, 0:512])
                nc.gpsimd.dma_start(out=wq_v, in_=wqT_r)
                nc.gpsimd.dma_start(out=xt0[:, 4:8, :], in_=xT_r[:, 4:8, 0:512])
                nc.gpsimd.dma_start(out=wk_v, in_=wkT_r)
                nc.gpsimd.dma_start(out=wv_v, in_=wvT_r)
                nc.gpsimd.dma_start(out=xt1[:, :, 0:512], in_=xT_r[:, :, 512:1024])
                nc.gpsimd.dma_start(out=xt1[:, :, 512:1024], in_=xT_r[:, :, 1024:1536])
                nc.gpsimd.dma_start(out=xt1[:, :, 1024:1536], in_=xT_r[:, :, 1536:S])
                for p in range(2):
                    nc.gpsimd.dma_start(out=wo_sb[p], in_=woT[p * 128:(p + 1) * 128, :])

                def xcols(c, a, b):
                    if b <= 512:
                        return xt0[:, c, a:b]
                    assert a >= 512
                    return xt1[:, c, a - 512:b - 512]

                def qk_chain(p, j, which, pool):
                    ps = pool.tile([128, 512], F32, tag="proj", name="ps")
                    w_v_ = wq_v if which == "q" else wk_v
                    for c in range(8):
                        nc.tensor.matmul(
                            ps, w_v_[:, c, p * 128:(p + 1) * 128],
                            xcols(c, j * 512, (j + 1) * 512),
                            start=(c == 0), stop=(c == 7))
                    if which == "q":
                        nc.vector.tensor_scalar_add(
                            qT_sb[p][:, j * 512:(j + 1) * 512], ps, bq_sb[:, p:p + 1])
                    else:
                        nc.vector.tensor_copy(kT_sb[p][:, j * 512:(j + 1) * 512], ps)

                def v_chain(t, pool):
                    ps_v = pool.tile([128, 256], F32, tag="proj", name="ps_v")
                    for c in range(8):
                        nc.tensor.matmul(
                            ps_v, xcols(c, t * 128, (t + 1) * 128), wv_v[:, c, :],
                            start=(c == 0), stop=(c == 7))
                    v_view = v_sb[t].rearrange("p (h w) -> p h w", w=65)
                    nc.vector.memset(v_view[:, :, 64:65], 1.0)
                    nc.vector.tensor_copy(
                        v_view[:, :, 0:64],
                        ps_v.rearrange("p (h w) -> p h w", w=64))

                norm_rest = []

                def emit_norm(p_, q0_, a_, b_, o_ps_, defer=True):
                    # copy values + sums out of PSUM for chunk columns [a, b);
                    # sums land on partition 0 (the DVE reciprocal mis-executes
                    # on HW with a partition-offset input)
                    w_ = b_ - a_
                    sums = rp.tile([1, 1024], F32, tag="sums", name="sums")
                    ocs = []
                    for s in range(2):
                        oc = rp.tile([64, 512], F32, tag=f"oc{s}", name="o_cp")
                        nc.vector.tensor_copy(oc[:, 0:w_], o_ps_[s][0:64, a_:b_])
                        nc.vector.tensor_copy(
                            sums[:, s * 512:s * 512 + w_],
                            o_ps_[s][64:65, a_:b_])
                        ocs.append(oc)
                    if defer:
                        norm_rest.append(
                            lambda: finish_norm(p_, q0_ + a_, w_, sums, ocs))
                    else:
                        finish_norm(p_, q0_ + a_, w_, sums, ocs)

                def finish_norm(p_, qa_, w_, sums, ocs):
                    rec = rp.tile([1, 1024], F32, tag="rec", name="recip")
                    for s in range(2):
                        nc.vector.reciprocal_approx_fast(
                            out=rec[:, s * 512:s * 512 + w_],
                            in_=sums[:, s * 512:s * 512 + w_])
                    bc = rp.tile([64, 1024], F32, tag="bc", name="bc")
                    nc.gpsimd.partition_broadcast(bc, rec)
                    for s in range(2):
                        nc.vector.tensor_mul(
                            outT_sb[p_][s * 64:(s + 1) * 64, qa_:qa_ + w_],
                            ocs[s][:, 0:w_],
                            bc[:, s * 512:s * 512 + w_])

                def emit_pair(p, fillers, stage=None):
                    # stage: (split_col, hookA, hookB) applied to the LAST
                    # chunk: columns [0, split) normalize right after their
                    # final PV (two k-tiles early), so most of the output
                    # projection tail overlaps the chunk's trailing k-tiles.
                    nchunks = len(CHUNKS[p])
                    for ci, (q0, qcw) in enumerate(CHUNKS[p]):
                        nkt = (q0 + qcw) // 128
                        last = stage is not None and ci == nchunks - 1
                        o_ps = [opp.tile([65, 512], F32, tag=f"o{s}", name=f"ops{s}")
                                for s in range(2)]
                        pend = None
                        for kt in range(nkt):
                            o = kt * 128 - q0
                            diag = o >= 0
                            lo = o if diag else 0
                            s_ab = sqp.tile([128, 1024], F32, tag="sq", name="s_ab")
                            s_v = s_ab.rearrange("p (c w) -> p c w", c=2)
                            for s in range(2):
                                nc.tensor.matmul(
                                    s_v[:, s, lo:qcw],
                                    kT_sb[p][s * 64:(s + 1) * 64,
                                             kt * 128:(kt + 1) * 128],
                                    qT_sb[p][s * 64:(s + 1) * 64,
                                             q0 + lo:q0 + qcw],
                                    start=True, stop=True,
                                    tile_position=(s * 64, 0),
                                    skip_group_check=True)
                            e_ab = ep.tile([128, 1024], F16, tag="e", name="e_ab")
                            e_v = e_ab.rearrange("p (c w) -> p c w", c=2)
                            nc.scalar.activation(
                                e_v[:, :, lo:qcw], s_v[:, :, lo:qcw],
                                AF.Exp, scale=0.125)
                            if diag:
                                nc.vector.tensor_mul(
                                    e_v[:, :, o:o + 128], e_v[:, :, o:o + 128],
                                    tri_v)
                            if kt == 0:
                                while norm_rest:
                                    norm_rest.pop(0)()
                            if fillers is not None:
                                fillers(ci, kt, nkt)
                            if pend is not None:
                                _kt, _e, _lo = pend
                                for s in range(2):
                                    hb = 2 * p + s
                                    nc.tensor.matmul(
                                        o_ps[s][:, _lo:qcw],
                                        v_sb[_kt][:, hb * 65:(hb + 1) * 65],
                                        _e[:, s, _lo:qcw],
                                        start=(_kt == 0), stop=False,
                                        skip_group_check=True)
                            pend = (kt, e_v, lo)
                            if last and kt == nkt - 2:
                                # PV(kt-1) just issued; chunk columns
                                # [0, split) are final in PSUM
                                split, hookA, _ = stage
                                emit_norm(p, q0, 0, split, o_ps, defer=False)
                                hookA()
                        _kt, _e, _lo = pend
                        for s in range(2):
                            hb = 2 * p + s
                            nc.tensor.matmul(
                                o_ps[s][:, _lo:qcw],
                                v_sb[_kt][:, hb * 65:(hb + 1) * 65],
                                _e[:, s, _lo:qcw],
                                start=False, stop=True,
                                skip_group_check=True)
                        if last:
                            split, _, hookB = stage
                            emit_norm(p, q0, split, qcw, o_ps, defer=False)
                            hookB()
                        else:
                            emit_norm(p, q0, 0, qcw, o_ps)
                        yield ci
                    while norm_rest:
                        norm_rest.pop(0)()

                # ---- pair 0 front: j0 q/k chains (v0-3 run as chunk-0 fillers)
                with tc.tile_pool(name="ppsA", bufs=2, space="PSUM") as ppsA:
                    ps_q0 = ppsA.tile([128, 512], F32, tag="projA", name="ps_q0")
                    ps_k0 = ppsA.tile([128, 512], F32, tag="projA", name="ps_k0")
                    for c in range(8):
                        nc.tensor.matmul(
                            ps_q0, wq_v[:, c, 0:128], xt0[:, c, :],
                            start=(c == 0), stop=(c == 7))
                    for c in range(8):
                        nc.tensor.matmul(
                            ps_k0, wk_v[:, c, 0:128], xt0[:, c, :],
                            start=(c == 0), stop=(c == 7))
                    nc.vector.tensor_scalar_add(
                        qT_sb[0][:, 0:512], ps_q0, bq_sb[:, 0:1])
                    nc.vector.tensor_copy(kT_sb[0][:, 0:512], ps_k0)
                with (
                    tc.tile_pool(name="sq", bufs=2, space="PSUM") as sqp,
                    tc.tile_pool(name="ops", bufs=1, space="PSUM") as opp,
                ):
                    with tc.tile_pool(name="pps", bufs=2, space="PSUM") as pps:
                        fillers = []
                        for j in range(1, 4):
                            fillers.append(lambda j=j: qk_chain(0, j, "q", pps))
                            fillers.append(lambda j=j: qk_chain(0, j, "k", pps))
                            for t in range(4 * j, 4 * j + 4):
                                fillers.append(lambda t=t: v_chain(t, pps))
                        # chunk 0 gets v0-3 + j1 + v4-7, chunk 1 j2+v8-11,
                        # chunk 2 j3+v12-15, chunk 3 all 8 pair-1 projections
                        v03 = [lambda t=t: v_chain(t, pps) for t in range(4)]
                        sched = {0: v03 + fillers[0:6], 1: fillers[6:12],
                                 2: fillers[12:18]}
                        sched[3] = []
                        for j in range(4):
                            sched[3].append(lambda j=j: qk_chain(1, j, "q", pps))
                            sched[3].append(lambda j=j: qk_chain(1, j, "k", pps))
                        queues = [list(sched.get(ci, [])) for ci in range(4)]

                        def filler_pop(ci, kt, nkt):
                            q = queues[ci]
                            rem_slots = nkt - kt
                            while q and len(q) >= rem_slots:
                                q.pop(0)()
                            if q:
                                q.pop(0)()

                        for _ci in emit_pair(0, filler_pop):
                            while queues[_ci]:
                                queues[_ci].pop(0)()

                    if debug:
                        for p in range(2):
                            nc.sync.dma_start(out=dbg["qT"][:, p, :], in_=qT_sb[p])
                            nc.sync.dma_start(out=dbg["kT"][:, p, :], in_=kT_sb[p])
                        for t in range(NKT):
                            nc.sync.dma_start(out=dbg["vv"][:, t, :], in_=v_sb[t])

                    # ---- pair 1: output projection as fillers ----
                    with tc.tile_pool(name="fps", bufs=2, space="PSUM") as fps:
                        with tc.tile_pool(name="fsb", bufs=4) as fsb:
                            c_alt = [0]

                            def c_unit(qt, oc_i, vec=False):
                                f_ps = fps.tile([128, 512], F32, tag="f", name="f_ps")
                                for p in range(2):
                                    nc.tensor.matmul(
                                        f_ps, outT_sb[p][:, qt * 128:(qt + 1) * 128],
                                        wo_sb[p][:, oc_i * 512:(oc_i + 1) * 512],
                                        start=(p == 0), stop=(p == 1))
                                f_sb = fsb.tile([128, 512], F16, tag="f", name="f_sb")
                                c_alt[0] ^= 1
                                if vec or c_alt[0]:
                                    nc.vector.tensor_copy(f_sb, f_ps)
                                else:
                                    nc.scalar.activation(f_sb, f_ps, AF.Identity)
                                nc.gpsimd.dma_start(
                                    out=y[qt * 128:(qt + 1) * 128,
                                          oc_i * 512:(oc_i + 1) * 512],
                                    in_=f_sb)

                            NCH = len(CHUNKS[1])
                            cqueues = [[] for _ in range(NCH)]

                            def c_pop(ci, kt, nkt):
                                q = cqueues[ci]
                                rem_slots = nkt - kt
                                while q and len(q) >= rem_slots:
                                    q.pop(0)()
                                if q:
                                    q.pop(0)()

                            qt_ranges = [(0, 4), (4, 8), (8, 12)]

                            def tail_hook_a():
                                # qt 12-13 project while k-tiles 14/15 finish
                                for qt in (12, 13):
                                    for oc_i in range(2):
                                        c_unit(qt, oc_i, vec=True)

                            def tail_hook_b():
                                for qt in (14, 15):
                                    for oc_i in range(2):
                                        c_unit(qt, oc_i, vec=True)

                            for ci in emit_pair(1, c_pop,
                                                stage=(256, tail_hook_a,
                                                       tail_hook_b)):
                                if ci < len(qt_ranges):
                                    a, b = qt_ranges[ci]
                                    units = []
                                    for qt in range(a, b):
                                        for oc_i in range(2):
                                            units.append(
                                                lambda qt=qt, oc_i=oc_i:
                                                c_unit(qt, oc_i))
                                    cqueues[ci + 1].extend(units)
                            for q in cqueues:
                                while q:
                                    q.pop(0)()

            if debug:
                for p in range(2):
                    nc.sync.dma_start(out=dbg["oT"][:, p, :], in_=outT_sb[p])

    nc.compile()
    return nc


_cached = {}


def _get_nc(debug=False):
    key = bool(debug)
    if key not in _cached:
        _cached[key] = _build(debug)
    return _cached[key]


def _prep_inputs(x, w_q, b_q, w_k, w_v):
    tri = np.triu(np.ones((128, 128), np.float32)).astype(NPF16)
    tri2 = np.concatenate([tri, tri], axis=1)
    wqT_f = np.ascontiguousarray(w_q.T).astype(NPF16)
    wkT_f = np.ascontiguousarray(w_k.T).astype(NPF16)
    wvT_f = np.ascontiguousarray(w_v.T).astype(NPF16)
    in_maps = []
    for core in range(N_CORES):
        b, hg = divmod(core, 4)
        cs = slice(hg * 256, (hg + 1) * 256)
        in_maps.append({
            "xT": np.ascontiguousarray(x[b].T).astype(NPF16),
            "wqT": np.ascontiguousarray(wqT_f[:, cs]),
            "wkT": np.ascontiguousarray(wkT_f[:, cs]),
            "wvT": np.ascontiguousarray(wvT_f[:, cs]),
            "bq2": np.ascontiguousarray(
                b_q[hg * 256:(hg + 1) * 256].reshape(2, 128).T.astype(np.float32)),
            "tri2": tri2,
        })
    return in_maps


def _numpy_reference(x, attention_mask, w_q, b_q, w_k, b_k, w_v, b_v, w_o, b_o):
    x = x.astype(np.float64)
    q = (x @ w_q.T + b_q).reshape(B, S, H, DK).transpose(0, 2, 1, 3)
    k = (x @ w_k.T + b_k).reshape(B, S, H, DK).transpose(0, 2, 1, 3)
    v = (x @ w_v.T + b_v).reshape(B, S, H, DK).transpose(0, 2, 1, 3)
    scores = np.einsum("bhqd,bhkd->bhqk", q, k, optimize=True) / np.sqrt(DK)
    causal = np.tril(np.ones((S, S), bool))
    mask = causal[None, None] & (attention_mask[:, None, None, :] != 0)
    scores = np.where(mask, scores, -np.inf)
    scores -= scores.max(-1, keepdims=True)
    e = np.exp(scores)
    attn = e / e.sum(-1, keepdims=True)
    out = np.einsum("bhqk,bhkd->bhqd", attn, v, optimize=True)
    out = out.transpose(0, 2, 1, 3).reshape(B, S, D)
    return (out @ w_o.T + b_o).astype(np.float32)


def kernel(x, attention_mask, w_q, b_q, w_k, b_k, w_v, b_v, w_o, b_o,
           _debug=False, _trace=False):
    x = np.asarray(x, np.float32)
    attention_mask = np.asarray(attention_mask)
    if not np.all(attention_mask != 0):
        return _numpy_reference(np.asarray(x), np.asarray(attention_mask),
                                *[np.asarray(a) for a in
                                  (w_q, b_q, w_k, b_k, w_v, b_v, w_o, b_o)])
    w_q, w_k, w_v, w_o = [np.asarray(w, np.float32) for w in (w_q, w_k, w_v, w_o)]
    b_q, b_k, b_v, b_o = [np.asarray(b, np.float32) for b in (b_q, b_k, b_v, b_o)]

    nc = _get_nc(_debug)
    in_maps = _prep_inputs(x, w_q, b_q, w_k, w_v)
    woT_f = np.ascontiguousarray(w_o.T).astype(NPF16)
    for core in range(N_CORES):
        hg = core % 4
        in_maps[core]["woT"] = np.ascontiguousarray(
            woT_f[hg * 256:(hg + 1) * 256, :])

    res = run_bass_kernel_spmd(nc, in_maps, list(range(N_CORES)), trace=_trace)
    const_row = (b_v @ w_o.T + b_o).astype(np.float32)
    y = np.zeros((B, S, D), np.float32)
    for core in range(N_CORES):
        b = core // 4
        y[b] += res.results[core]["y"].astype(np.float32)
    y += const_row
    if _debug or _trace:
        return y, res
    return y
